# revision 36
# baseline (speedup 1.0000x reference)
"""CTC loss kernel for Trainium2 (8 NeuronCores, data-parallel over batch).

Math: with raw logits G[b,t,s] = pred[b,t,ext[b,s]] (ext = blank-interleaved
targets) the CTC forward recursion commutes with the per-frame log-softmax
normalizer: running the recursion on raw logits and subtracting
sum_t logsumexp_c(pred[b,t,:]) at the end gives the same loss.

Device work, per core (16 samples):
1. sum_c exp(pred) per (b,t): fp8(e4m3) pred streamed via HWDGE, ACT
   exp+accumulate (ACT-throughput bound; fp8 host-cast keeps the DMA side
   far under the SBUF fabric share).
2. The CTC recursion in probability space, forward and backward chains in
   lockstep columns of one [102, 32] state: per super-step one PE matmul
   z = W^T X (W = [K1; K2] encodes the +0/+1/+2 state shifts, identical for
   both chains because the backward state is stored index-reversed) and two
   DVE multiplies by host-prepacked exp'd logits. Per-frame max-logit folding
   (host) replaces renormalization entirely; host compensates exactly.
Host finishes the tiny join + scalar math in float64.
"""

import sys

sys.path.insert(0, "/opt/trn_rl_repo")

import ml_dtypes
import numpy as np

import concourse.bacc as bacc
import concourse.tile as tile
from concourse import mybir
from concourse.bass_utils import run_bass_kernel_spmd

B, T, C, L = 128, 160, 6625, 25
S = 2 * L + 1  # 51 CTC states
SB2 = 2 * S  # 102: [y; yq] stacked state rows
N_CORES = 8
BS = B // N_CORES  # 16 samples per core
NCOL = 2 * BS  # 32: fwd cols 0-15, bwd cols 16-31
HS = T // 2  # 80 frames per chain
NSUP = HS - 1  # 79 super-steps
TBLK = 8  # t-values per 128-row streaming block (8*16 = 128 rows)
NBLK = T // TBLK  # 20
# one accumulator column per streaming block (every block is one ACT
# exp+accumulate; fp8 DMAs are fast enough that head/tail chunking no
# longer pays). The whole stream is fp8: ACT runs at 1 elem/cycle/lane
# regardless of dtype (apparent dtype effects in earlier traces were the
# device clock varying 1.0 vs 1.2 GHz between runs), so fewer DMA bytes
# is strictly better.
NACC = NBLK
NEG = -1.0e4  # exp() underflows to exactly 0.0f
BOOST = 0.5  # per-frame fold = fmax - BOOST: keeps fp32 range centered

f32 = mybir.dt.float32
f16 = mybir.dt.float16
f8 = mybir.dt.float8e4
Exp = mybir.ActivationFunctionType.Exp
np_f8 = ml_dtypes.float8_e4m3

_CACHE = {}
_HOST = {}


def _build_program():
    if "nc" in _CACHE:
        return _CACHE["nc"]
    nc = bacc.Bacc("TRN2", target_bir_lowering=False, debug=False,
                   num_devices=N_CORES)
    pred_d = nc.dram_tensor("pred", [BS, T, C], f8, kind="ExternalInput").ap()
    w_d = nc.dram_tensor("w", [SB2, SB2], f32, kind="ExternalInput").ap()
    x0_d = nc.dram_tensor("x0", [SB2, NCOL], f32, kind="ExternalInput").ap()
    pq_d = nc.dram_tensor("pq", [SB2, NSUP * NCOL], f32,
                          kind="ExternalInput").ap()
    acc_d = nc.dram_tensor("acc", [128, NACC], f32,
                           kind="ExternalOutput").ap()
    xf_d = nc.dram_tensor("xfin", [SB2, NCOL], f32,
                          kind="ExternalOutput").ap()

    # pq column chunks (whole super-steps per chunk) for early recursion start
    PQCH = []
    st = 0
    for n in (20, 20, 20, 19):
        PQCH.append((st * NCOL, (st + n) * NCOL))
        st += n

    with tile.TileContext(nc) as tc:
        with (
            tc.tile_pool(name="persist", bufs=1) as pp,
            tc.tile_pool(name="sm8", bufs=6) as sm8,
            tc.tile_pool(name="zp", bufs=4, space="PSUM") as psp,
        ):
            # ---- recursion inputs (host pre-exponentiates pq/x0, so ACT
            # does zero recursion work). Only pq chunk 0 loads ahead of the
            # stream; chunks 1-3 are interleaved behind the first stream
            # blocks in the sync queue (the recursion consumes chunk k only
            # ~20k us in).
            wt = pp.tile([SB2, SB2], f32, tag="wt")
            pqe = pp.tile([SB2, NSUP * NCOL], f32, tag="pqe")
            Xa = pp.tile([SB2, NCOL], f32, tag="Xa")
            Xb = pp.tile([SB2, NCOL], f32, tag="Xb")
            # block 0 rides first on the sync ring (its completion gates the
            # ACT stream start); the small recursion inputs follow, and the
            # pq pack goes via the idle SWDGE queue (all before the
            # recursion's DVE ops start, so no shared-port contention)
            ct0 = sm8.tile([128, C], f8, tag="mid8")
            nc.sync.dma_start(out=ct0[:], in_=pred_d[:, 0:TBLK, :])
            nc.sync.dma_start(out=Xa[:], in_=x0_d[:])
            nc.sync.dma_start(out=wt[:], in_=w_d[:])
            for a, b in PQCH:
                nc.gpsimd.dma_start(out=pqe[:, a:b], in_=pq_d[:, a:b])

            # ---- streaming sum(exp(pred)) over C, 128 (b,t) rows per block.
            # pred is pre-cast to fp8 e4m3 on the host; all loads ride the
            # sync HWDGE ring (SWDGE would stall: its Q7 descriptor writes
            # arbitrate for the DVE/GpSimd shared SBUF port pair that the
            # recursion DVE holds most of the time; HWDGE is immune). The
            # stream is ACT-throughput bound (1 elem/cycle/lane), so the DMA
            # side has ample slack. exp output goes to one reused fp16
            # scratch (ACT engine port, free); accumulation stays fp32.
            # Every accumulate targets its own column of one persistent tile
            # and the whole accumulator ships in a single DMA at the end.
            accA = pp.tile([128, NACC], f32, tag="accA")
            scr = pp.tile([128, C], f16, tag="scr")

            def stream_block(j):
                ct = sm8.tile([128, C], f8, tag="mid8")
                nc.sync.dma_start(out=ct[:],
                                  in_=pred_d[:, j * TBLK:(j + 1) * TBLK, :])
                nc.scalar.activation(scr[:], ct[:], Exp,
                                     accum_out=accA[:, j:j + 1])

            nc.scalar.activation(scr[:], ct0[:], Exp,
                                 accum_out=accA[:, 0:1])
            for j in range(1, 4):
                stream_block(j)

            # ---- lockstep fwd/bwd recursion: 79 x (1 matmul + 1 DVE mul).
            # The stationary matrix is [W | W] so the matmul lands z
            # duplicated on partitions 0-50 and 51-101 — DVE lanes cannot
            # cross partitions, so the y- and yq-halves each need z in their
            # own partitions; the duplicate makes the whole state update a
            # single partition-aligned multiply. (Emitted after the first
            # three stream blocks so every pqe range is written, in program
            # order, before the step that reads it.)
            cur, nxt = Xa, Xb
            for i in range(NSUP):
                z = psp.tile([SB2, NCOL], f32, tag="z")
                nc.tensor.matmul(z[:], wt[:], cur[:])
                c0 = i * NCOL
                nc.vector.tensor_mul(out=nxt[:], in0=z[:],
                                     in1=pqe[:, c0:c0 + NCOL])
                cur, nxt = nxt, cur

            for j in range(4, NBLK):
                stream_block(j)
            nc.sync.dma_start(out=acc_d[:], in_=accA[:])
            # recursion result ships via the otherwise-idle SWDGE queue: on
            # the sync ring the scheduler hoists it ahead of later stream
            # DMAs and its recursion-end wait head-of-line-blocks them
            nc.gpsimd.dma_start(out=xf_d[:], in_=cur[:])

    nc.compile()
    _CACHE["nc"] = nc
    return nc


def prepare_in_maps(pred, targets, lens):
    """Host prep: gathered+folded logit packs, per-core sharding."""
    ext = np.zeros((B, S), dtype=np.int64)
    ext[:, 1::2] = targets
    G = pred[np.arange(B)[:, None, None], np.arange(T)[None, :, None],
             ext[:, None, :]]  # [B, T, S]
    valid = np.arange(S)[None, :] < (2 * lens + 1)[:, None]  # [B, S]
    G = np.where(valid[:, None, :], G, NEG).astype(np.float32)
    skip = np.pad((ext[:, 2:] != ext[:, :-2]) & (ext[:, 2:] != 0),
                  ((0, 0), (2, 0)))  # [B,S] bool: s-2 -> s allowed
    fmax = G.max(2) - BOOST  # [B,T] per-frame fold
    _HOST["fmax_sum"] = fmax.sum(1)  # [B] exact compensation
    Gh = G - fmax[:, :, None]
    # fwd yq mask (yq[s] = y[s]*skip_ok[s+2]); bwd mask in reversed coords
    skf = np.full((B, S), NEG, np.float32)
    skf[:, :S - 2] = np.where(skip[:, 2:], 0.0, NEG)
    skb = np.where(skip[:, ::-1], 0.0, NEG).astype(np.float32)
    term = np.full((B, S), NEG, np.float32)
    term[np.arange(B), 2 * lens] = 0.0
    term[np.arange(B), 2 * lens - 1] = 0.0
    im = np.full((S,), NEG, np.float32)
    im[:2] = 0.0
    y0f = Gh[:, 0, :] + im[None, :]  # [B,S] alpha_0 logits
    y0b = (Gh[:, T - 1, :] + term)[:, ::-1]  # gamma_{T-1}, reversed s

    Wm = np.zeros((SB2, S), np.float32)  # z[f] = y[f] + y[f-1] + yq[f-2]
    for f in range(S):
        Wm[f, f] = 1.0
        if f >= 1:
            Wm[f - 1, f] = 1.0
        if f >= 2:
            Wm[S + f - 2, f] = 1.0
    Wm = np.concatenate([Wm, Wm], axis=1)  # duplicate z onto both halves

    pred8 = pred.astype(np_f8)
    in_maps = []
    for c in range(N_CORES):
        sl = slice(c * BS, (c + 1) * BS)
        Ghf = Gh[sl, 1:HS, :]  # [16,79,S] fwd frames t=1..79
        Ghb = Gh[sl, T - 2:HS - 1:-1, ::-1]  # [16,79,S] t=158..80, rev s
        skfc, skbc = skf[sl], skb[sl]
        x0 = np.empty((SB2, NCOL), np.float32)
        x0[0:S, 0:BS] = y0f[sl].T
        x0[0:S, BS:] = y0b[sl].T
        x0[S:, 0:BS] = (y0f[sl] + skfc).T
        x0[S:, BS:] = (y0b[sl] + skbc).T
        pq = np.empty((SB2, NSUP, NCOL), np.float32)
        pq[0:S, :, 0:BS] = Ghf.transpose(2, 1, 0)
        pq[0:S, :, BS:] = Ghb.transpose(2, 1, 0)
        pq[S:, :, 0:BS] = (Ghf + skfc[:, None, :]).transpose(2, 1, 0)
        pq[S:, :, BS:] = (Ghb + skbc[:, None, :]).transpose(2, 1, 0)
        in_maps.append({
            "pred": np.ascontiguousarray(pred8[sl]),
            "w": Wm,
            "x0": np.exp(x0),
            "pq": np.exp(np.ascontiguousarray(
                pq.reshape(SB2, NSUP * NCOL))),
        })
    return in_maps


def finish_host(results, lens):
    """Combine per-core outputs into the scalar mean loss (float64)."""
    fmax_sum = _HOST["fmax_sum"]
    loss_b = np.zeros(B, dtype=np.float64)
    with np.errstate(divide="ignore", invalid="ignore"):
        for c in range(N_CORES):
            r = results[c]
            acc = r["acc"].astype(np.float64)  # [128, NACC]
            ssum = acc.T  # [NBLK, 128] per-block row sums; row = b*8+t_off
            lse = np.log(ssum)  # [NBLK, 128]
            s_lse = lse.reshape(NBLK, BS, TBLK).sum((0, 2))  # [BS]
            xf = r["xfin"].astype(np.float64)  # [SB2, NCOL]
            a79 = xf[0:S, 0:BS]  # [S,16] alpha_79
            g80 = xf[0:S, BS:][::-1, :]  # gamma_80[s]
            gq80 = xf[S:, BS:][::-1, :]  # gamma_80[s]*skip_ok[s]
            beta = g80.copy()
            beta[:-1] += g80[1:]
            beta[:-2] += gq80[2:]
            P = (a79 * beta).sum(0)  # [16]
            sl = slice(c * BS, (c + 1) * BS)
            logP = np.log(P) + fmax_sum[sl]
            loss_b[sl] = s_lse - logP
    loss_b = np.where(loss_b >= 1e29, 0.0, loss_b)
    loss_b = np.where(np.isfinite(loss_b), loss_b, 0.0)
    loss = np.mean(loss_b / np.maximum(lens.astype(np.float64), 1.0))
    return np.float32(loss)


def kernel(pred, targets, targets_lengths):
    pred = np.asarray(pred, dtype=np.float32)
    targets = np.asarray(targets).astype(np.int64)
    lens = np.asarray(targets_lengths).astype(np.int64)

    nc = _build_program()
    in_maps = prepare_in_maps(pred, targets, lens)
    res = run_bass_kernel_spmd(nc, in_maps, core_ids=list(range(N_CORES)))
    return finish_host(res.results, lens)
